# revision 24
# baseline (speedup 1.0000x reference)
"""Bilinear grid-sample (align_corners=True, border-clamped) on Trainium2.

Problem: x [8, 64, 256, 256] f32, grid [8, 256, 256, 2] f32 in [-1, 1]
         -> out [8, 64, 256, 256] f32.

Sharding: pure data-parallel over batch N=8 -> one sample per NeuronCore.

Per-core layout strategy (host-side marshalling only; all arithmetic that
is part of the operator runs on-device):
  - x[n] is fed as a "vertically paired" channels-last slab vp [H*W, 4*C]:
    row p holds the full 2x2 bilinear patch anchored at pixel p =
    y0*W + x0, i.e. [x(y0,x0,:), x(y0,x0+1,:), x(y0+1,x0,:),
    x(y0+1,x0+1,:)] (1KB). One output pixel therefore needs exactly ONE
    contiguous 1KB gather read.
  - gathers use the GPSIMD dma_gather custom DMA (int16 indices => 32768
    addressable 1KB rows = half the image). Pixels are binned by their y0
    image half (top: y0 < H/2, bottom: y0 >= H/2); rel = y0*W + x0 -
    (half ? H*W/2 : 0) fits int16 exactly, and the two halves differ only
    in the static base offset of the source access pattern.
  - the host bins pixel ids into [top..., pad, bottom..., pad] where pads
    are synthetic corner pixels; every gather call (slot) is homogeneous
    in half, so its base is static. Binning uses the same fp32 operations
    the device uses, and the device clamps rel to [0, 32767] anyway.
  - the device computes indices/weights, gathers, combines (weighted sum
    of the 4 patch corners), and stores results in stream order; the host
    permutes rows back and transposes to [C, H, W].

dma_gather stream mapping (hardware-verified): stream position i takes
its int16 index from idx_tile[i % 16, i // 16] (partitions 0-15, plus the
replicas at partitions 16-31 for queue 0's second Q7 core) and writes the
gathered row to partition i % 128, slot i // 128. Partition p therefore
owns stream positions == p (mod 128); the grid is fed in that layout, and
a small DRAM bounce rearranges the computed rel16 into the 16-partition
wrapped layout.
"""

import numpy as np

import concourse.bacc as bacc
import concourse.bass as bass
import concourse.tile as tile
from concourse import bass_utils, mybir

N, C, H, W = 8, 64, 256, 256

F32 = mybir.dt.float32
I32 = mybir.dt.int32
I16 = mybir.dt.int16
BF16 = mybir.dt.bfloat16
AF = mybir.ActivationFunctionType
OP = mybir.AluOpType
P = 128


def build_program(
    h=H, w=W, c=C, k=16, sA=17, sB=17, gbufs=3, mbufs=2, rbufs=3, repeat=1,
    nqueues=2, do_gather=True, do_combine=True, do_store=True,
    vdt=F32, single_packet=False, trim=False, do_wrapped=True, lg=1,
    cmaj=False, slab=4, zero_g=False, red=False, padd=False,
):
    """Per-core Bass program for sA top-half + sB bottom-half gather slots.

    Each slot covers 128*k stream positions; the gather fetches one 4*c
    value (1KB f32 / 512B bf16) patch row per position from the vertically-
    paired slab. repeat > 1 re-emits the whole pipeline for repeat-delta
    timing.

    vdt: dtype of slab / gathered data / combine / output.
    cmaj: slab rows are channel-major [(4 corners) per channel] so the
        combine multiply reads the gathered tile at unit stride.
    single_packet: coalesce each gather's descriptors into one packet.
    trim: feed a host validity mask and emit -1 indices for pad stream
        positions so the gather ucode trims them (pads are binned at the
        tail of each half, hence trailing within their gather call).
    lg: pixels served per gather descriptor ("run length"). The host
        packs each run's lg pixels into lg consecutive sub-slots of one
        partition; one descriptor fetches the consecutive slab rows that
        cover them, landing exactly where the combine expects them. SWDGE
        ucode desc-gen is the bottleneck, so fewer descriptors win.
    slab: 4 = one slab row per pixel holds the full 2x2 patch [4c]; a run
        of lg pixels fetches the lg rows [v, v+lg) (lg*4c values).
        2 = one slab row per pixel holds the vertical pair [x(y0,x0,:),
        x(y0+1,x0,:)] [2c]; a run of lg pixels fetches lg+1 rows
        ((lg+1)*2c values) and pixel s reads the overlapping 4c window at
        row offset s, corners ordered (TL, BL, TR, BR). Nearly halves
        gather bytes per pixel at equal descriptor count.
    """
    npix = h * w
    halfbase = (h // 2) * w          # patch-row offset of the bottom half
    S = sA + sB
    qp = S * k                        # stream positions per partition
    scx = (w - 1) / 2.0
    scy = (h - 1) / 2.0
    relmax = float(halfbase + w - 1)  # clamp bound; == 32767 at full size
    e4 = 4 * c                        # one pixel's 2x2 patch: 4 corners x c
    assert slab in (2, 4)
    assert not (slab == 2 and trim)
    e_row = (slab // 2) * 2 * c       # slab row width in elements
    fr = lg + 1 if slab == 2 else lg  # slab rows fetched per descriptor
    gsz = fr * e_row                  # gathered elements per run
    assert k % lg == 0 and (P * k // lg) % 128 == 0

    nc = bacc.Bacc(trn_type="TRN2", debug=False, num_swdge_queues=nqueues)
    vp = nc.dram_tensor("vp", [npix + fr, e_row], vdt, kind="ExternalInput")
    gridf = nc.dram_tensor("gridf", [P, qp * 2], F32, kind="ExternalInput")
    validf = (
        nc.dram_tensor("validf", [P, qp], F32, kind="ExternalInput")
        if trim else None
    )
    cntf = (
        nc.dram_tensor("cntf", [1, S], I32, kind="ExternalInput")
        if trim else None
    )
    bounces = [
        nc.dram_tensor(f"bounce{r}", [P * qp], I16, kind="Internal")
        for r in range(repeat)
    ]
    odt = F32 if red else vdt         # reduce-combine must emit f32
    outdev = nc.dram_tensor("outdev", [P, qp * c], odt, kind="ExternalOutput")

    bases = {
        "A": bass.AP(vp, 0, [[e_row, npix], [1, gsz]]),
        "B": bass.AP(
            vp, halfbase * e_row, [[e_row, npix - halfbase], [1, gsz]]
        ),
    }
    kg = k // lg                      # gather rows (runs) per partition-slot

    with tile.TileContext(nc) as tc:
      for rep in range(repeat):
        with tc.tile_pool(name=f"persist{rep}", bufs=1) as persist:
            w4 = persist.tile([P, qp * 4], vdt, tag="w4")  # (w00,w01,w10,w11)
            wrapped = persist.tile([P, S * kg * 8], I16, tag="wrapped")
            if trim:
                cntt = persist.tile([1, S], I32, tag="cntt")
                nc.sync.dma_start(cntt[:], cntf[:])

            with tc.tile_pool(name=f"prep{rep}", bufs=1) as prep:
                gridt = prep.tile([P, qp * 2], F32, tag="gridt")
                nc.sync.dma_start(gridt[:], gridf[:])
                if trim:
                    validt = prep.tile([P, qp], F32, tag="validt")
                    nc.sync.dma_start(validt[:], validf[:])
                g3 = gridt[:].rearrange("p (q xy) -> p q xy", xy=2)

                def sc_mul_add(src, scale, name):
                    # (src + 1) * scale with the reference's rounding order,
                    # as two explicitly rounded DVE ops (host replicates it)
                    a = prep.tile([P, qp], F32, tag=f"sma_{name}")
                    nc.vector.tensor_scalar_add(a[:], src, 1.0)
                    nc.vector.tensor_scalar_mul(a[:], a[:], scale)
                    return a

                ix = sc_mul_add(g3[:, :, 0], scx, "x")
                iy = sc_mul_add(g3[:, :, 1], scy, "y")

                def floor_and_fracs(v, name):
                    vi = prep.tile([P, qp], I32, tag=f"vi_{name}")
                    nc.vector.tensor_copy(vi[:], v)
                    vf = prep.tile([P, qp], F32, tag=f"vf_{name}")
                    nc.vector.tensor_copy(vf[:], vi[:])
                    m = prep.tile([P, qp], F32, tag=f"m_{name}")
                    nc.vector.tensor_tensor(m[:], vf[:], v, op=OP.is_gt)
                    nc.vector.tensor_tensor(vf[:], vf[:], m[:], op=OP.subtract)
                    fr1 = prep.tile([P, qp], F32, tag=f"fr1_{name}")
                    nc.vector.tensor_tensor(fr1[:], v, vf[:], op=OP.subtract)
                    fr0 = prep.tile([P, qp], F32, tag=f"fr0_{name}")
                    nc.scalar.activation(fr0[:], fr1[:], AF.Copy, bias=1.0, scale=-1.0)
                    return vf, fr0, fr1

                x0f, wx0, wx1 = floor_and_fracs(ix[:], "x")
                y0f, wy0, wy1 = floor_and_fracs(iy[:], "y")

                # corner order: slab4 row is (TL, TR, BL, BR); slab2 window
                # is (TL, BL, TR, BR) — x-column-major pairs
                w44 = w4[:].rearrange("p (q x) -> p q x", x=4)
                worder = (
                    ((wy0, wx0), (wy1, wx0), (wy0, wx1), (wy1, wx1))
                    if slab == 2 else
                    ((wy0, wx0), (wy0, wx1), (wy1, wx0), (wy1, wx1))
                )
                for xi, (wyi, wxj) in enumerate(worder):
                    nc.vector.tensor_tensor(
                        w44[:, :, xi], wyi[:], wxj[:], op=OP.mult
                    )

                # rel = y0*w + x0 - halfbase*(y0 >= h/2), clamped to int16 range
                mhalf = prep.tile([P, qp], F32, tag="mhalf")
                nc.vector.tensor_scalar(
                    mhalf[:], y0f[:], float(h // 2), None, op0=OP.is_ge
                )
                idxf = prep.tile([P, qp], F32, tag="idxf")
                nc.vector.scalar_tensor_tensor(
                    idxf[:], in0=y0f[:], scalar=float(w), in1=x0f[:],
                    op0=OP.mult, op1=OP.add,
                )
                relf = prep.tile([P, qp], F32, tag="relf")
                nc.vector.scalar_tensor_tensor(
                    relf[:], in0=mhalf[:], scalar=-float(halfbase), in1=idxf[:],
                    op0=OP.mult, op1=OP.add,
                )
                nc.vector.tensor_scalar_max(relf[:], relf[:], 0.0)
                nc.vector.tensor_scalar_min(relf[:], relf[:], relmax)
                if trim:
                    # rel' = (rel + 1) * valid - 1: pads (valid=0) become -1
                    # so the gather ucode trims them off the stream tail.
                    nc.vector.scalar_tensor_tensor(
                        relf[:], in0=relf[:], scalar=1.0, in1=validt[:],
                        op0=OP.add, op1=OP.mult,
                    )
                    nc.vector.tensor_scalar_add(relf[:], relf[:], -1.0)
                rel16 = prep.tile([P, qp], I16, tag="rel16")
                nc.vector.tensor_copy(rel16[:], relf[:])

                # bounce through DRAM into the 16-partition wrapped layout,
                # replicated to partition groups 0 and 1 (queue 0's Q7 pair)
                bounce = bounces[rep]
                nc.vector.memset(wrapped[:], 0)
                if do_wrapped:
                    nc.sync.dma_start(
                        bounce[:].rearrange("(p q) -> p q", p=P), rel16[:]
                    )
                    wr4 = wrapped[:].rearrange(
                        "p (t b1 b0) -> p t b1 b0", t=S, b1=kg, b0=8
                    )
                    src4 = bass.AP(
                        bounce, 0, [[qp, 16], [k, S], [lg, kg], [16 * qp, 8]]
                    )
                    for g in range(2 * nqueues):
                        nc.sync.dma_start(wr4[16 * g : 16 * (g + 1)], src4)

            with (
                tc.tile_pool(name=f"g{rep}", bufs=gbufs) as gp,
                tc.tile_pool(name=f"m{rep}", bufs=mbufs) as mp,
                tc.tile_pool(name=f"r{rep}", bufs=rbufs) as rp,
            ):
                if (trim and do_gather) or zero_g:
                    # trimmed gathers leave pad rows unwritten; zero each
                    # buffer once so stale reads stay finite. zero_g also
                    # pre-zeros for no-gather ablations (avoids denormal-
                    # poisoned DVE timing on uninitialized SBUF).
                    for _ in range(gbufs):
                        gz = gp.tile([P, kg * gsz], vdt, tag="g0")
                        nc.vector.memset(gz[:], 0)
                for t in range(S):
                    half = "A" if t < sA else "B"
                    qs = slice(t * k, (t + 1) * k)
                    idxs = wrapped[:, t * kg * 8 : (t + 1) * kg * 8]

                    g0 = gp.tile([P, kg * gsz], vdt, tag="g0")
                    if do_gather:
                        nreg = (
                            nc.gpsimd.value_load(
                                cntt[0:1, t : t + 1], min_val=1, max_val=P * kg
                            )
                            if trim else P * kg
                        )
                        nc.gpsimd.dma_gather(
                            out_ap=g0[:].rearrange("p (s e) -> p s e", e=gsz),
                            in_ap=bases[half], idxs_ap=idxs,
                            num_idxs=P * kg, num_idxs_reg=nreg,
                            elem_size=gsz, elem_step=e_row,
                            single_packet=single_packet,
                            queue_num=t % nqueues,
                        )
                    else:
                        nc.vector.memset(g0[:1, :1], 0)

                    if not do_combine and not do_store:
                        continue
                    r = rp.tile([P, k * c], odt, tag="r")
                    if do_combine and slab == 2:
                        # pixel (run j, sub s) reads the 4c window at row
                        # offset s: rows s (TL,BL) and s+1 (TR,BR). Two
                        # shifted non-overlapping views replace a single
                        # overlapping one. cmaj rows interleave (T,B) per
                        # channel so every operand's innermost dim is the
                        # stride-1 corner pair (DVE bf16 fast mode).
                        g0v = g0[:].rearrange(
                            "p (j rr e) -> p j rr e", rr=fr, e=e_row
                        )
                        w4s = w4[:].rearrange("p (q x) -> p q x", x=4)
                        w4s = w4s[:, qs, :].rearrange(
                            "p (j s) x -> p j s x", j=kg
                        )
                        ma = mp.tile([P, k * 2 * c], vdt, tag="m0")
                        mb = mp.tile([P, k * 2 * c], vdt, tag="a")
                        if cmaj:
                            gA = g0v[:, :, 0:lg, :].rearrange(
                                "p j s (ch x) -> p j s ch x", x=2
                            )
                            gB = g0v[:, :, 1:fr, :].rearrange(
                                "p j s (ch x) -> p j s ch x", x=2
                            )
                            wA = w4s[:, :, :, 0:2].unsqueeze(3).to_broadcast(
                                [P, kg, lg, c, 2]
                            )
                            wB = w4s[:, :, :, 2:4].unsqueeze(3).to_broadcast(
                                [P, kg, lg, c, 2]
                            )
                            mav = ma[:].rearrange(
                                "p (j s ch x) -> p j s ch x", j=kg, s=lg, x=2
                            )
                            mbv = mb[:].rearrange(
                                "p (j s ch x) -> p j s ch x", j=kg, s=lg, x=2
                            )
                        else:
                            gA = g0v[:, :, 0:lg, :].rearrange(
                                "p j s (x ch) -> p j s x ch", x=2
                            )
                            gB = g0v[:, :, 1:fr, :].rearrange(
                                "p j s (x ch) -> p j s x ch", x=2
                            )
                            wA = w4s[:, :, :, 0:2].unsqueeze(4).to_broadcast(
                                [P, kg, lg, 2, c]
                            )
                            wB = w4s[:, :, :, 2:4].unsqueeze(4).to_broadcast(
                                [P, kg, lg, 2, c]
                            )
                            mav = ma[:].rearrange(
                                "p (j s x ch) -> p j s x ch", j=kg, s=lg, x=2
                            )
                            mbv = mb[:].rearrange(
                                "p (j s x ch) -> p j s x ch", j=kg, s=lg, x=2
                            )
                        nc.vector.tensor_tensor(mav, gA, wA, op=OP.mult)
                        nc.vector.tensor_tensor(mbv, gB, wB, op=OP.mult)
                        nc.vector.tensor_tensor(
                            ma[:], ma[:], mb[:], op=OP.add
                        )
                        if cmaj:
                            maq = ma[:].rearrange("p (f x) -> p f x", x=2)
                            nc.vector.tensor_tensor(
                                r[:], maq[:, :, 0], maq[:, :, 1], op=OP.add
                            )
                        else:
                            maq = ma[:].rearrange(
                                "p (f x ch) -> p f x ch", x=2, ch=c
                            )
                            nc.vector.tensor_tensor(
                                r[:].rearrange("p (f ch) -> p f ch", ch=c),
                                maq[:, :, 0, :], maq[:, :, 1, :], op=OP.add,
                            )
                    elif do_combine:
                        # weighted corners; iterate (slot, channel, corner)
                        m0 = mp.tile([P, k * e4], vdt, tag="m0")
                        m0v = m0[:].rearrange(
                            "p (s ch x) -> p s ch x", s=k, ch=c, x=4
                        )
                        if cmaj:
                            g0v = g0[:].rearrange(
                                "p (s ch x) -> p s ch x", ch=c, x=4
                            )
                        else:
                            g0v = g0[:].rearrange(
                                "p (s x ch) -> p s ch x", x=4, ch=c
                            )
                        w4v = w4[:].rearrange("p (q x) -> p q x", x=4)[:, qs, :]
                        w4v = w4v.unsqueeze(2).to_broadcast([P, k, c, 4])
                        nc.vector.tensor_tensor(m0v, g0v, w4v, op=OP.mult)

                        if red:
                            # fused corner reduction (cmaj: x innermost)
                            assert cmaj
                            nc.vector.tensor_reduce(
                                r[:], m0v, axis=mybir.AxisListType.X,
                                op=OP.add,
                            )
                        else:
                            # y-add: m[..., 0:2] + m[..., 2:4]
                            a = mp.tile([P, k * c * 2], vdt, tag="a")
                            m0q = m0[:].rearrange("p (f x) -> p f x", x=2)
                            nc.vector.tensor_tensor(
                                a[:].rearrange("p (f x) -> p f x", x=2),
                                m0q[:, 0::2, :], m0q[:, 1::2, :], op=OP.add,
                            )
                            av = a[:].rearrange("p (f x) -> p f x", x=2)
                            # the x-add reads stride-2 views (DVE 1x mode);
                            # padd runs it on the otherwise-idle GPSIMD so
                            # it overlaps the next slot's DVE mult/y-add
                            eng = nc.gpsimd if padd else nc.vector
                            eng.tensor_tensor(
                                r[:], av[:, :, 0], av[:, :, 1], op=OP.add
                            )
                    else:
                        nc.vector.tensor_copy(r[:], g0[:, : k * c])

                    if do_store:
                        nc.sync.dma_start(
                            outdev[:, t * k * c : (t + 1) * k * c], r[:]
                        )

    nc.compile()
    return nc


def _host_yhalf(grid_n, h, w):
    """Bit-exact replica of the device's y0 computation -> bottom-half mask."""
    gy = np.asarray(grid_n, np.float32).reshape(-1, 2)[:, 1]
    scy = np.float32((h - 1) / 2.0)
    iy = ((gy + np.float32(1.0)).astype(np.float32) * scy).astype(np.float32)
    y0 = np.floor(iy)
    return y0 >= (h // 2)


def _host_rel(grid_n, h, w):
    """Bit-exact replica of the device's rel (patch-row index) computation."""
    g = np.asarray(grid_n, np.float32).reshape(-1, 2)
    scx = np.float32((w - 1) / 2.0)
    scy = np.float32((h - 1) / 2.0)
    ix = ((g[:, 0] + np.float32(1.0)).astype(np.float32) * scx).astype(np.float32)
    iy = ((g[:, 1] + np.float32(1.0)).astype(np.float32) * scy).astype(np.float32)
    x0 = np.floor(ix)
    y0 = np.floor(iy)
    rel = y0 * np.float32(w) + x0
    rel = rel - np.float32((h // 2) * w) * (y0 >= (h // 2))
    return np.clip(rel, 0, (h // 2) * w + w - 1).astype(np.int64)


def _build_vp(x_n, h, w, c, lg=1, cmaj=False, slab=4):
    """Gather slab, one row per pixel p.

    slab=2: vertical-pair row [x(y,x,:), x(y+1,x,:)] (2c wide); a run's
    descriptor fetches lg+1 consecutive rows.
    slab=4, cmaj=False: corner-major row [TL c..., TR c..., BL c..., BR c...].
    slab=4, cmaj=True: channel-major row [(TL,TR,BL,BR) for ch0, ch1, ...] so
    the combine's (slot, channel, corner) iteration is unit-stride.
    Extra rows are appended so gathers anchored at the last valid row stay
    in-bounds.
    """
    npix = h * w
    xt = np.zeros((npix + 2 * w + 2, c), np.float32)
    xt[:npix] = np.asarray(x_n, np.float32).reshape(c, npix).T
    if slab == 2:
        nr = npix + lg + 1
        vp = np.empty((nr, 2 * c), np.float32)
        if cmaj:
            v2 = vp.reshape(nr, c, 2)
            v2[:, :, 0] = xt[:nr]
            v2[:, :, 1] = xt[w : w + nr]
        else:
            vp[:, :c] = xt[:nr]
            vp[:, c:] = xt[w : w + nr]
        return vp
    if cmaj:
        vp4 = np.empty((npix + lg, c, 4), np.float32)
        vp4[:, :, 0] = xt[: npix + lg]
        vp4[:, :, 1] = xt[1 : npix + lg + 1]
        vp4[:, :, 2] = xt[w : w + npix + lg]
        vp4[:, :, 3] = xt[w + 1 : w + 1 + npix + lg]
        return vp4.reshape(npix + lg, 4 * c)
    flat = xt.reshape(-1)
    pair = np.lib.stride_tricks.as_strided(
        flat, shape=(npix + w + lg, 2 * c), strides=(4 * c, 4)
    )
    vp = np.empty((npix + lg, 4 * c), np.float32)
    vp[:, : 2 * c] = pair[: npix + lg]
    vp[:, 2 * c :] = pair[w : w + npix + lg]
    return vp


def _pack_runs(ids, rel, lg):
    """Greedy-pack pixels (patch row rel[j] for pixel ids[j]) into runs of
    lg consecutive rows. Returns int64 [nruns, lg] of pixel ids (-1 = pad);
    sub-position s of a run with base row v holds a pixel with rel == v+s.
    Sub-position 0 is always real."""
    order = np.argsort(rel, kind="stable")
    sid = np.asarray(ids)[order]
    srel = np.asarray(rel)[order]
    if lg == 1:
        return sid[:, None]
    n = len(sid)
    vals, starts = np.unique(srel, return_index=True)
    counts = np.diff(np.append(starts, n))
    taken = np.zeros(len(vals), np.int64)
    val2g = np.full(int(vals[-1]) + lg + 1, -1, np.int64)
    val2g[vals] = np.arange(len(vals))
    out = []
    gi = 0
    total = n
    while total > 0:
        while taken[gi] >= counts[gi]:
            gi += 1
        v = int(vals[gi])
        row = np.full(lg, -1, np.int64)
        for s in range(lg):
            g2 = val2g[v + s] if v + s < len(val2g) else -1
            if g2 >= 0 and taken[g2] < counts[g2]:
                row[s] = sid[starts[g2] + taken[g2]]
                taken[g2] += 1
                total -= 1
        out.append(row)
    return np.asarray(out, np.int64).reshape(-1, lg)


def prep_core(x_n, grid_n, sA, sB, h=H, w=W, c=C, k=16, np_vdt=np.float32,
              trim=False, sort=False, lg=1, packs=None, cmaj=False, slab=4):
    """Marshal one sample: slab, run-packed grid feed, unpermute metadata.

    Runs of lg pixels with consecutive patch rows are packed into lg
    consecutive sub-slots of one partition; run i' of a half maps to
    slot t = i'//(P*kg), partition p = i'%P, sub-slot j = (i'%(P*kg))//P
    (kg = k//lg), occupying grid columns q = t*k + j*lg + s.
    """
    npix = h * w
    S = sA + sB
    qp = S * k
    kg = k // lg
    rslot = P * kg                    # runs per slot

    vp = _build_vp(x_n, h, w, c, lg=lg, cmaj=cmaj, slab=slab)
    if np_vdt is not np.float32:
        vp = vp.astype(np_vdt)

    grid_flat = np.asarray(grid_n, np.float32).reshape(npix, 2)
    if packs is None:
        mB = _host_yhalf(grid_n, h, w)
        rel = _host_rel(grid_n, h, w)
        packs = tuple(
            _pack_runs(ids, rel[ids], lg)
            for ids in (np.flatnonzero(~mB), np.flatnonzero(mB))
        )
    runsA, runsB = packs
    assert len(runsA) <= sA * rslot and len(runsB) <= sB * rslot

    # run-major member table covering every (run-slot, sub) position
    members = np.full((S * rslot, lg), -1, np.int64)
    members[: len(runsA)] = runsA
    members[sA * rslot : sA * rslot + len(runsB)] = runsB
    rvalid = np.zeros(S * rslot, np.float32)
    rvalid[: len(runsA)] = 1.0
    rvalid[sA * rslot : sA * rslot + len(runsB)] = 1.0

    # stream position of (run i', sub s): q = t*k + j*lg + s, p = i'%P
    ri = np.arange(S * rslot)
    rt = ri // rslot
    rp = ri % P
    rj = (ri % rslot) // P
    q0 = rt * k + rj * lg             # column of sub 0

    gridfeed = np.empty((P, qp, 2), np.float32)
    gridfeed[:, :, 0] = -1.0          # pad: top-left corner (y0 = 0, x0 = 0)
    gridfeed[:, :, 1] = -1.0
    # bottom-half pads: y0 well inside the bottom half so rel stays in-range
    bcols = (np.arange(qp) // k) >= sA
    gridfeed[:, bcols, 1] = 0.9
    validfeed = np.zeros((P, qp), np.float32)
    rows = np.empty(npix, np.int64)
    for s in range(lg):
        m = members[:, s]
        sel = m >= 0
        gridfeed[rp[sel], q0[sel] + s] = grid_flat[m[sel]]
        rows[m[sel]] = rp[sel] * qp + q0[sel] + s
    # validity is read at run-base columns only
    validfeed[rp[rvalid > 0], q0[rvalid > 0]] = 1.0

    im = {"vp": vp, "gridf": np.ascontiguousarray(gridfeed.reshape(P, qp * 2))}
    if trim:
        cnt = rvalid.reshape(S, rslot).sum(1).astype(np.int32)
        for t in np.flatnonzero(cnt == 0):
            # every gather call needs >= 1 untrimmed idx; promote the first
            # pad run of the slot (its gather lands in an unused row)
            validfeed[0, t * k : t * k + lg] = 1.0
            cnt[t] = 1
        im["validf"] = np.ascontiguousarray(validfeed)
        im["cntf"] = cnt.reshape(1, S)
    return im, rows


_PROGRAMS = {}


def get_program(sA, sB, **kw):
    key = (sA, sB, tuple(sorted(kw.items())))
    if key not in _PROGRAMS:
        _PROGRAMS[key] = build_program(sA=sA, sB=sB, **kw)
    return _PROGRAMS[key]


# chosen configuration (see ablations): bf16 values, full-patch slab
# (slab=4) channel-major so the combine multiply is unit-stride (DVE bf16
# fast mode), lg=12 run-packed gathers (6KB descriptor per run of up to 12
# pixels with consecutive patch rows). Descriptor generation is cheap
# (~0.3ns/desc); the costs are ~12ns/descriptor of DMA-pipeline overhead
# plus per-position DVE combine work, balanced around lg=12.
# nqueues=4 regresses (measured); trim=True exhausts sequencer registers.
LGDEF = 12
KGDEF = 2
KDEF = KGDEF * LGDEF
CONFIG = dict(vdt=BF16, lg=LGDEF, cmaj=True, trim=False, k=KDEF, slab=4,
              nqueues=1, gbufs=3)
NP_VDT = np.dtype(mybir.dt.np(BF16))


def plan_shards(grid, k=KDEF, lg=LGDEF):
    """Pack runs for every sample and derive the shared slot counts."""
    kg = k // lg
    rslot = P * kg
    packs = []
    for n in range(grid.shape[0]):
        mB = _host_yhalf(grid[n], H, W)
        rel = _host_rel(grid[n], H, W)
        idsA, idsB = np.flatnonzero(~mB), np.flatnonzero(mB)
        packs.append((_pack_runs(idsA, rel[idsA], lg),
                      _pack_runs(idsB, rel[idsB], lg)))
    sA = max(-(-len(pA) // rslot) for pA, _ in packs)
    sB = max(-(-len(pB) // rslot) for _, pB in packs)
    return packs, sA, sB


def kernel(x, grid):
    x = np.asarray(x, np.float32)
    grid = np.asarray(grid, np.float32)
    assert x.shape == (N, C, H, W) and grid.shape == (N, H, W, 2)

    packs, sA, sB = plan_shards(grid)
    nc = get_program(sA, sB, **CONFIG)

    in_maps, rowmaps = [], []
    for n in range(N):
        im, rows = prep_core(x[n], grid[n], sA, sB, k=KDEF, np_vdt=NP_VDT,
                             trim=CONFIG["trim"], lg=LGDEF, packs=packs[n],
                             cmaj=CONFIG["cmaj"], slab=CONFIG["slab"])
        in_maps.append(im)
        rowmaps.append(rows)

    res = bass_utils.run_bass_kernel_spmd(nc, in_maps, core_ids=list(range(N)))

    out = np.empty((N, C, H, W), np.float32)
    for n in range(N):
        od = np.asarray(res.results[n]["outdev"]).astype(np.float32)
        od = od.reshape(P * (sA + sB) * KDEF, C)
        out[n] = od[rowmaps[n]].T.reshape(C, H, W)
    return out



# revision 25
# speedup vs baseline: 1.4286x; 1.4286x over previous
"""Bilinear grid-sample (align_corners=True, border-clamped) on Trainium2.

Problem: x [8, 64, 256, 256] f32, grid [8, 256, 256, 2] f32 in [-1, 1]
         -> out [8, 64, 256, 256] f32.

Sharding: pure data-parallel over batch N=8 -> one sample per NeuronCore.

Per-core layout strategy (host-side marshalling only; all arithmetic that
is part of the operator runs on-device):
  - x[n] is fed as a "vertically paired" channels-last slab vp [H*W, 4*C]:
    row p holds the full 2x2 bilinear patch anchored at pixel p =
    y0*W + x0, i.e. [x(y0,x0,:), x(y0,x0+1,:), x(y0+1,x0,:),
    x(y0+1,x0+1,:)] (1KB). One output pixel therefore needs exactly ONE
    contiguous 1KB gather read.
  - gathers use the GPSIMD dma_gather custom DMA (int16 indices => 32768
    addressable 1KB rows = half the image). Pixels are binned by their y0
    image half (top: y0 < H/2, bottom: y0 >= H/2); rel = y0*W + x0 -
    (half ? H*W/2 : 0) fits int16 exactly, and the two halves differ only
    in the static base offset of the source access pattern.
  - the host bins pixel ids into [top..., pad, bottom..., pad] where pads
    are synthetic corner pixels; every gather call (slot) is homogeneous
    in half, so its base is static. Binning uses the same fp32 operations
    the device uses, and the device clamps rel to [0, 32767] anyway.
  - the device computes indices/weights, gathers, combines (weighted sum
    of the 4 patch corners), and stores results in stream order; the host
    permutes rows back and transposes to [C, H, W].

dma_gather stream mapping (hardware-verified): stream position i takes
its int16 index from idx_tile[i % 16, i // 16] (partitions 0-15, plus the
replicas at partitions 16-31 for queue 0's second Q7 core) and writes the
gathered row to partition i % 128, slot i // 128. Partition p therefore
owns stream positions == p (mod 128); the grid is fed in that layout, and
a small DRAM bounce rearranges the computed rel16 into the 16-partition
wrapped layout.
"""

import numpy as np

import concourse.bacc as bacc
import concourse.bass as bass
import concourse.tile as tile
from concourse import bass_utils, mybir

N, C, H, W = 8, 64, 256, 256

F32 = mybir.dt.float32
I32 = mybir.dt.int32
I16 = mybir.dt.int16
BF16 = mybir.dt.bfloat16
AF = mybir.ActivationFunctionType
OP = mybir.AluOpType
P = 128


def build_program(
    h=H, w=W, c=C, k=16, sA=17, sB=17, gbufs=3, mbufs=2, rbufs=3, repeat=1,
    nqueues=2, do_gather=True, do_combine=True, do_store=True,
    vdt=F32, single_packet=False, trim=False, do_wrapped=True, lg=1,
    cmaj=False, slab=4, zero_g=False, red=False, padd=False,
):
    """Per-core Bass program for sA top-half + sB bottom-half gather slots.

    Each slot covers 128*k stream positions; the gather fetches one 4*c
    value (1KB f32 / 512B bf16) patch row per position from the vertically-
    paired slab. repeat > 1 re-emits the whole pipeline for repeat-delta
    timing.

    vdt: dtype of slab / gathered data / combine / output.
    cmaj: slab rows are channel-major [(4 corners) per channel] so the
        combine multiply reads the gathered tile at unit stride.
    single_packet: coalesce each gather's descriptors into one packet.
    trim: feed a host validity mask and emit -1 indices for pad stream
        positions so the gather ucode trims them (pads are binned at the
        tail of each half, hence trailing within their gather call).
    lg: pixels served per gather descriptor ("run length"). The host
        packs each run's lg pixels into lg consecutive sub-slots of one
        partition; one descriptor fetches the consecutive slab rows that
        cover them, landing exactly where the combine expects them. SWDGE
        ucode desc-gen is the bottleneck, so fewer descriptors win.
    slab: 4 = one slab row per pixel holds the full 2x2 patch [4c]; a run
        of lg pixels fetches the lg rows [v, v+lg) (lg*4c values).
        2 = one slab row per pixel holds the vertical pair [x(y0,x0,:),
        x(y0+1,x0,:)] [2c]; a run of lg pixels fetches lg+1 rows
        ((lg+1)*2c values) and pixel s reads the overlapping 4c window at
        row offset s, corners ordered (TL, BL, TR, BR). Nearly halves
        gather bytes per pixel at equal descriptor count.
    """
    npix = h * w
    halfbase = (h // 2) * w          # patch-row offset of the bottom half
    S = sA + sB
    qp = S * k                        # stream positions per partition
    scx = (w - 1) / 2.0
    scy = (h - 1) / 2.0
    relmax = float(halfbase + w - 1)  # clamp bound; == 32767 at full size
    e4 = 4 * c                        # one pixel's 2x2 patch: 4 corners x c
    assert slab in (2, 4)
    assert not (slab == 2 and trim)
    e_row = (slab // 2) * 2 * c       # slab row width in elements
    fr = lg + 1 if slab == 2 else lg  # slab rows fetched per descriptor
    gsz = fr * e_row                  # gathered elements per run
    assert k % lg == 0 and (P * k // lg) % 128 == 0

    nc = bacc.Bacc(trn_type="TRN2", debug=False, num_swdge_queues=nqueues)
    vp = nc.dram_tensor("vp", [npix + fr, e_row], vdt, kind="ExternalInput")
    gridf = nc.dram_tensor("gridf", [P, qp * 2], F32, kind="ExternalInput")
    validf = (
        nc.dram_tensor("validf", [P, qp], F32, kind="ExternalInput")
        if trim else None
    )
    cntf = (
        nc.dram_tensor("cntf", [1, S], I32, kind="ExternalInput")
        if trim else None
    )
    bounces = [
        nc.dram_tensor(f"bounce{r}", [P * qp], I16, kind="Internal")
        for r in range(repeat)
    ]
    odt = F32 if red else vdt         # reduce-combine must emit f32
    outdev = nc.dram_tensor("outdev", [P, qp * c], odt, kind="ExternalOutput")

    bases = {
        "A": bass.AP(vp, 0, [[e_row, npix], [1, gsz]]),
        "B": bass.AP(
            vp, halfbase * e_row, [[e_row, npix - halfbase], [1, gsz]]
        ),
    }
    kg = k // lg                      # gather rows (runs) per partition-slot

    with tile.TileContext(nc) as tc:
      for rep in range(repeat):
        with tc.tile_pool(name=f"persist{rep}", bufs=1) as persist:
            w4 = persist.tile([P, qp * 4], vdt, tag="w4")  # (w00,w01,w10,w11)
            wrapped = persist.tile([P, S * kg * 8], I16, tag="wrapped")
            if trim:
                cntt = persist.tile([1, S], I32, tag="cntt")
                nc.sync.dma_start(cntt[:], cntf[:])

            with tc.tile_pool(name=f"prep{rep}", bufs=1) as prep:
                gridt = prep.tile([P, qp * 2], F32, tag="gridt")
                nc.sync.dma_start(gridt[:], gridf[:])
                if trim:
                    validt = prep.tile([P, qp], F32, tag="validt")
                    nc.sync.dma_start(validt[:], validf[:])
                g3 = gridt[:].rearrange("p (q xy) -> p q xy", xy=2)

                def sc_mul_add(src, scale, name):
                    # (src + 1) * scale with the reference's rounding order,
                    # as two explicitly rounded DVE ops (host replicates it)
                    a = prep.tile([P, qp], F32, tag=f"sma_{name}")
                    nc.vector.tensor_scalar_add(a[:], src, 1.0)
                    nc.vector.tensor_scalar_mul(a[:], a[:], scale)
                    return a

                ix = sc_mul_add(g3[:, :, 0], scx, "x")
                iy = sc_mul_add(g3[:, :, 1], scy, "y")

                def floor_and_fracs(v, name):
                    vi = prep.tile([P, qp], I32, tag=f"vi_{name}")
                    nc.vector.tensor_copy(vi[:], v)
                    vf = prep.tile([P, qp], F32, tag=f"vf_{name}")
                    nc.vector.tensor_copy(vf[:], vi[:])
                    m = prep.tile([P, qp], F32, tag=f"m_{name}")
                    nc.vector.tensor_tensor(m[:], vf[:], v, op=OP.is_gt)
                    nc.vector.tensor_tensor(vf[:], vf[:], m[:], op=OP.subtract)
                    fr1 = prep.tile([P, qp], F32, tag=f"fr1_{name}")
                    nc.vector.tensor_tensor(fr1[:], v, vf[:], op=OP.subtract)
                    fr0 = prep.tile([P, qp], F32, tag=f"fr0_{name}")
                    nc.scalar.activation(fr0[:], fr1[:], AF.Copy, bias=1.0, scale=-1.0)
                    return vf, fr0, fr1

                x0f, wx0, wx1 = floor_and_fracs(ix[:], "x")
                y0f, wy0, wy1 = floor_and_fracs(iy[:], "y")

                # corner order: slab4 row is (TL, TR, BL, BR); slab2 window
                # is (TL, BL, TR, BR) — x-column-major pairs
                w44 = w4[:].rearrange("p (q x) -> p q x", x=4)
                worder = (
                    ((wy0, wx0), (wy1, wx0), (wy0, wx1), (wy1, wx1))
                    if slab == 2 else
                    ((wy0, wx0), (wy0, wx1), (wy1, wx0), (wy1, wx1))
                )
                for xi, (wyi, wxj) in enumerate(worder):
                    nc.vector.tensor_tensor(
                        w44[:, :, xi], wyi[:], wxj[:], op=OP.mult
                    )

                # rel = y0*w + x0 - halfbase*(y0 >= h/2), clamped to int16 range
                mhalf = prep.tile([P, qp], F32, tag="mhalf")
                nc.vector.tensor_scalar(
                    mhalf[:], y0f[:], float(h // 2), None, op0=OP.is_ge
                )
                idxf = prep.tile([P, qp], F32, tag="idxf")
                nc.vector.scalar_tensor_tensor(
                    idxf[:], in0=y0f[:], scalar=float(w), in1=x0f[:],
                    op0=OP.mult, op1=OP.add,
                )
                relf = prep.tile([P, qp], F32, tag="relf")
                nc.vector.scalar_tensor_tensor(
                    relf[:], in0=mhalf[:], scalar=-float(halfbase), in1=idxf[:],
                    op0=OP.mult, op1=OP.add,
                )
                nc.vector.tensor_scalar_max(relf[:], relf[:], 0.0)
                nc.vector.tensor_scalar_min(relf[:], relf[:], relmax)
                if trim:
                    # rel' = (rel + 1) * valid - 1: pads (valid=0) become -1
                    # so the gather ucode trims them off the stream tail.
                    nc.vector.scalar_tensor_tensor(
                        relf[:], in0=relf[:], scalar=1.0, in1=validt[:],
                        op0=OP.add, op1=OP.mult,
                    )
                    nc.vector.tensor_scalar_add(relf[:], relf[:], -1.0)
                rel16 = prep.tile([P, qp], I16, tag="rel16")
                nc.vector.tensor_copy(rel16[:], relf[:])

                # bounce through DRAM into the 16-partition wrapped layout,
                # replicated to partition groups 0 and 1 (queue 0's Q7 pair)
                bounce = bounces[rep]
                nc.vector.memset(wrapped[:], 0)
                if do_wrapped:
                    nc.sync.dma_start(
                        bounce[:].rearrange("(p q) -> p q", p=P), rel16[:]
                    )
                    wr4 = wrapped[:].rearrange(
                        "p (t b1 b0) -> p t b1 b0", t=S, b1=kg, b0=8
                    )
                    src4 = bass.AP(
                        bounce, 0, [[qp, 16], [k, S], [lg, kg], [16 * qp, 8]]
                    )
                    for g in range(2 * nqueues):
                        nc.sync.dma_start(wr4[16 * g : 16 * (g + 1)], src4)

            with (
                tc.tile_pool(name=f"g{rep}", bufs=gbufs) as gp,
                tc.tile_pool(name=f"m{rep}", bufs=mbufs) as mp,
                tc.tile_pool(name=f"r{rep}", bufs=rbufs) as rp,
            ):
                if (trim and do_gather) or zero_g:
                    # trimmed gathers leave pad rows unwritten; zero each
                    # buffer once so stale reads stay finite. zero_g also
                    # pre-zeros for no-gather ablations (avoids denormal-
                    # poisoned DVE timing on uninitialized SBUF).
                    for _ in range(gbufs):
                        gz = gp.tile([P, kg * gsz], vdt, tag="g0")
                        nc.vector.memset(gz[:], 0)
                for t in range(S):
                    half = "A" if t < sA else "B"
                    qs = slice(t * k, (t + 1) * k)
                    idxs = wrapped[:, t * kg * 8 : (t + 1) * kg * 8]

                    g0 = gp.tile([P, kg * gsz], vdt, tag="g0")
                    if do_gather:
                        nreg = (
                            nc.gpsimd.value_load(
                                cntt[0:1, t : t + 1], min_val=1, max_val=P * kg
                            )
                            if trim else P * kg
                        )
                        nc.gpsimd.dma_gather(
                            out_ap=g0[:].rearrange("p (s e) -> p s e", e=gsz),
                            in_ap=bases[half], idxs_ap=idxs,
                            num_idxs=P * kg, num_idxs_reg=nreg,
                            elem_size=gsz, elem_step=e_row,
                            single_packet=single_packet,
                            queue_num=t % nqueues,
                        )
                    else:
                        nc.vector.memset(g0[:1, :1], 0)

                    if not do_combine and not do_store:
                        continue
                    r = rp.tile([P, k * c], odt, tag="r")
                    if do_combine and slab == 2:
                        # pixel (run j, sub s) reads the 4c window at row
                        # offset s: rows s (TL,BL) and s+1 (TR,BR). Two
                        # shifted non-overlapping views replace a single
                        # overlapping one. cmaj rows interleave (T,B) per
                        # channel so every operand's innermost dim is the
                        # stride-1 corner pair (DVE bf16 fast mode).
                        g0v = g0[:].rearrange(
                            "p (j rr e) -> p j rr e", rr=fr, e=e_row
                        )
                        w4s = w4[:].rearrange("p (q x) -> p q x", x=4)
                        w4s = w4s[:, qs, :].rearrange(
                            "p (j s) x -> p j s x", j=kg
                        )
                        ma = mp.tile([P, k * 2 * c], vdt, tag="m0")
                        mb = mp.tile([P, k * 2 * c], vdt, tag="a")
                        if cmaj:
                            gA = g0v[:, :, 0:lg, :].rearrange(
                                "p j s (ch x) -> p j s ch x", x=2
                            )
                            gB = g0v[:, :, 1:fr, :].rearrange(
                                "p j s (ch x) -> p j s ch x", x=2
                            )
                            wA = w4s[:, :, :, 0:2].unsqueeze(3).to_broadcast(
                                [P, kg, lg, c, 2]
                            )
                            wB = w4s[:, :, :, 2:4].unsqueeze(3).to_broadcast(
                                [P, kg, lg, c, 2]
                            )
                            mav = ma[:].rearrange(
                                "p (j s ch x) -> p j s ch x", j=kg, s=lg, x=2
                            )
                            mbv = mb[:].rearrange(
                                "p (j s ch x) -> p j s ch x", j=kg, s=lg, x=2
                            )
                        else:
                            gA = g0v[:, :, 0:lg, :].rearrange(
                                "p j s (x ch) -> p j s x ch", x=2
                            )
                            gB = g0v[:, :, 1:fr, :].rearrange(
                                "p j s (x ch) -> p j s x ch", x=2
                            )
                            wA = w4s[:, :, :, 0:2].unsqueeze(4).to_broadcast(
                                [P, kg, lg, 2, c]
                            )
                            wB = w4s[:, :, :, 2:4].unsqueeze(4).to_broadcast(
                                [P, kg, lg, 2, c]
                            )
                            mav = ma[:].rearrange(
                                "p (j s x ch) -> p j s x ch", j=kg, s=lg, x=2
                            )
                            mbv = mb[:].rearrange(
                                "p (j s x ch) -> p j s x ch", j=kg, s=lg, x=2
                            )
                        nc.vector.tensor_tensor(mav, gA, wA, op=OP.mult)
                        nc.vector.tensor_tensor(mbv, gB, wB, op=OP.mult)
                        nc.vector.tensor_tensor(
                            ma[:], ma[:], mb[:], op=OP.add
                        )
                        if cmaj:
                            maq = ma[:].rearrange("p (f x) -> p f x", x=2)
                            nc.vector.tensor_tensor(
                                r[:], maq[:, :, 0], maq[:, :, 1], op=OP.add
                            )
                        else:
                            maq = ma[:].rearrange(
                                "p (f x ch) -> p f x ch", x=2, ch=c
                            )
                            nc.vector.tensor_tensor(
                                r[:].rearrange("p (f ch) -> p f ch", ch=c),
                                maq[:, :, 0, :], maq[:, :, 1, :], op=OP.add,
                            )
                    elif do_combine:
                        # weighted corners; iterate (slot, channel, corner)
                        m0 = mp.tile([P, k * e4], vdt, tag="m0")
                        m0v = m0[:].rearrange(
                            "p (s ch x) -> p s ch x", s=k, ch=c, x=4
                        )
                        if cmaj:
                            g0v = g0[:].rearrange(
                                "p (s ch x) -> p s ch x", ch=c, x=4
                            )
                        else:
                            g0v = g0[:].rearrange(
                                "p (s x ch) -> p s ch x", x=4, ch=c
                            )
                        w4v = w4[:].rearrange("p (q x) -> p q x", x=4)[:, qs, :]
                        w4v = w4v.unsqueeze(2).to_broadcast([P, k, c, 4])
                        nc.vector.tensor_tensor(m0v, g0v, w4v, op=OP.mult)

                        if red:
                            # fused corner reduction (cmaj: x innermost)
                            assert cmaj
                            nc.vector.tensor_reduce(
                                r[:], m0v, axis=mybir.AxisListType.X,
                                op=OP.add,
                            )
                        else:
                            # y-add: m[..., 0:2] + m[..., 2:4]
                            a = mp.tile([P, k * c * 2], vdt, tag="a")
                            m0q = m0[:].rearrange("p (f x) -> p f x", x=2)
                            nc.vector.tensor_tensor(
                                a[:].rearrange("p (f x) -> p f x", x=2),
                                m0q[:, 0::2, :], m0q[:, 1::2, :], op=OP.add,
                            )
                            av = a[:].rearrange("p (f x) -> p f x", x=2)
                            # the x-add reads stride-2 views (DVE 1x mode);
                            # padd runs it on the otherwise-idle GPSIMD so
                            # it overlaps the next slot's DVE mult/y-add
                            eng = nc.gpsimd if padd else nc.vector
                            eng.tensor_tensor(
                                r[:], av[:, :, 0], av[:, :, 1], op=OP.add
                            )
                    else:
                        nc.vector.tensor_copy(r[:], g0[:, : k * c])

                    if do_store:
                        nc.sync.dma_start(
                            outdev[:, t * k * c : (t + 1) * k * c], r[:]
                        )

    nc.compile()
    return nc


def _host_yhalf(grid_n, h, w):
    """Bit-exact replica of the device's y0 computation -> bottom-half mask."""
    gy = np.asarray(grid_n, np.float32).reshape(-1, 2)[:, 1]
    scy = np.float32((h - 1) / 2.0)
    iy = ((gy + np.float32(1.0)).astype(np.float32) * scy).astype(np.float32)
    y0 = np.floor(iy)
    return y0 >= (h // 2)


def _host_rel(grid_n, h, w):
    """Bit-exact replica of the device's rel (patch-row index) computation."""
    g = np.asarray(grid_n, np.float32).reshape(-1, 2)
    scx = np.float32((w - 1) / 2.0)
    scy = np.float32((h - 1) / 2.0)
    ix = ((g[:, 0] + np.float32(1.0)).astype(np.float32) * scx).astype(np.float32)
    iy = ((g[:, 1] + np.float32(1.0)).astype(np.float32) * scy).astype(np.float32)
    x0 = np.floor(ix)
    y0 = np.floor(iy)
    rel = y0 * np.float32(w) + x0
    rel = rel - np.float32((h // 2) * w) * (y0 >= (h // 2))
    return np.clip(rel, 0, (h // 2) * w + w - 1).astype(np.int64)


def _build_vp(x_n, h, w, c, lg=1, cmaj=False, slab=4):
    """Gather slab, one row per pixel p.

    slab=2: vertical-pair row [x(y,x,:), x(y+1,x,:)] (2c wide); a run's
    descriptor fetches lg+1 consecutive rows.
    slab=4, cmaj=False: corner-major row [TL c..., TR c..., BL c..., BR c...].
    slab=4, cmaj=True: channel-major row [(TL,TR,BL,BR) for ch0, ch1, ...] so
    the combine's (slot, channel, corner) iteration is unit-stride.
    Extra rows are appended so gathers anchored at the last valid row stay
    in-bounds.
    """
    npix = h * w
    xt = np.zeros((npix + 2 * w + 2, c), np.float32)
    xt[:npix] = np.asarray(x_n, np.float32).reshape(c, npix).T
    if slab == 2:
        nr = npix + lg + 1
        vp = np.empty((nr, 2 * c), np.float32)
        if cmaj:
            v2 = vp.reshape(nr, c, 2)
            v2[:, :, 0] = xt[:nr]
            v2[:, :, 1] = xt[w : w + nr]
        else:
            vp[:, :c] = xt[:nr]
            vp[:, c:] = xt[w : w + nr]
        return vp
    if cmaj:
        vp4 = np.empty((npix + lg, c, 4), np.float32)
        vp4[:, :, 0] = xt[: npix + lg]
        vp4[:, :, 1] = xt[1 : npix + lg + 1]
        vp4[:, :, 2] = xt[w : w + npix + lg]
        vp4[:, :, 3] = xt[w + 1 : w + 1 + npix + lg]
        return vp4.reshape(npix + lg, 4 * c)
    flat = xt.reshape(-1)
    pair = np.lib.stride_tricks.as_strided(
        flat, shape=(npix + w + lg, 2 * c), strides=(4 * c, 4)
    )
    vp = np.empty((npix + lg, 4 * c), np.float32)
    vp[:, : 2 * c] = pair[: npix + lg]
    vp[:, 2 * c :] = pair[w : w + npix + lg]
    return vp


def _pack_runs(ids, rel, lg):
    """Greedy-pack pixels (patch row rel[j] for pixel ids[j]) into runs of
    lg consecutive rows. Returns int64 [nruns, lg] of pixel ids (-1 = pad);
    sub-position s of a run with base row v holds a pixel with rel == v+s.
    Sub-position 0 is always real."""
    order = np.argsort(rel, kind="stable")
    sid = np.asarray(ids)[order]
    srel = np.asarray(rel)[order]
    if lg == 1:
        return sid[:, None]
    n = len(sid)
    vals, starts = np.unique(srel, return_index=True)
    counts = np.diff(np.append(starts, n))
    taken = np.zeros(len(vals), np.int64)
    val2g = np.full(int(vals[-1]) + lg + 1, -1, np.int64)
    val2g[vals] = np.arange(len(vals))
    out = []
    gi = 0
    total = n
    while total > 0:
        while taken[gi] >= counts[gi]:
            gi += 1
        v = int(vals[gi])
        row = np.full(lg, -1, np.int64)
        for s in range(lg):
            g2 = val2g[v + s] if v + s < len(val2g) else -1
            if g2 >= 0 and taken[g2] < counts[g2]:
                row[s] = sid[starts[g2] + taken[g2]]
                taken[g2] += 1
                total -= 1
        out.append(row)
    return np.asarray(out, np.int64).reshape(-1, lg)


def prep_core(x_n, grid_n, sA, sB, h=H, w=W, c=C, k=16, np_vdt=np.float32,
              trim=False, sort=False, lg=1, packs=None, cmaj=False, slab=4):
    """Marshal one sample: slab, run-packed grid feed, unpermute metadata.

    Runs of lg pixels with consecutive patch rows are packed into lg
    consecutive sub-slots of one partition; run i' of a half maps to
    slot t = i'//(P*kg), partition p = i'%P, sub-slot j = (i'%(P*kg))//P
    (kg = k//lg), occupying grid columns q = t*k + j*lg + s.
    """
    npix = h * w
    S = sA + sB
    qp = S * k
    kg = k // lg
    rslot = P * kg                    # runs per slot

    vp = _build_vp(x_n, h, w, c, lg=lg, cmaj=cmaj, slab=slab)
    if np_vdt is not np.float32:
        vp = vp.astype(np_vdt)

    grid_flat = np.asarray(grid_n, np.float32).reshape(npix, 2)
    if packs is None:
        mB = _host_yhalf(grid_n, h, w)
        rel = _host_rel(grid_n, h, w)
        packs = tuple(
            _pack_runs(ids, rel[ids], lg)
            for ids in (np.flatnonzero(~mB), np.flatnonzero(mB))
        )
    runsA, runsB = packs
    assert len(runsA) <= sA * rslot and len(runsB) <= sB * rslot

    # run-major member table covering every (run-slot, sub) position
    members = np.full((S * rslot, lg), -1, np.int64)
    members[: len(runsA)] = runsA
    members[sA * rslot : sA * rslot + len(runsB)] = runsB
    rvalid = np.zeros(S * rslot, np.float32)
    rvalid[: len(runsA)] = 1.0
    rvalid[sA * rslot : sA * rslot + len(runsB)] = 1.0

    # stream position of (run i', sub s): q = t*k + j*lg + s, p = i'%P
    ri = np.arange(S * rslot)
    rt = ri // rslot
    rp = ri % P
    rj = (ri % rslot) // P
    q0 = rt * k + rj * lg             # column of sub 0

    gridfeed = np.empty((P, qp, 2), np.float32)
    gridfeed[:, :, 0] = -1.0          # pad: top-left corner (y0 = 0, x0 = 0)
    gridfeed[:, :, 1] = -1.0
    # bottom-half pads: y0 well inside the bottom half so rel stays in-range
    bcols = (np.arange(qp) // k) >= sA
    gridfeed[:, bcols, 1] = 0.9
    validfeed = np.zeros((P, qp), np.float32)
    rows = np.empty(npix, np.int64)
    for s in range(lg):
        m = members[:, s]
        sel = m >= 0
        gridfeed[rp[sel], q0[sel] + s] = grid_flat[m[sel]]
        rows[m[sel]] = rp[sel] * qp + q0[sel] + s
    # validity is read at run-base columns only
    validfeed[rp[rvalid > 0], q0[rvalid > 0]] = 1.0

    im = {"vp": vp, "gridf": np.ascontiguousarray(gridfeed.reshape(P, qp * 2))}
    if trim:
        cnt = rvalid.reshape(S, rslot).sum(1).astype(np.int32)
        for t in np.flatnonzero(cnt == 0):
            # every gather call needs >= 1 untrimmed idx; promote the first
            # pad run of the slot (its gather lands in an unused row)
            validfeed[0, t * k : t * k + lg] = 1.0
            cnt[t] = 1
        im["validf"] = np.ascontiguousarray(validfeed)
        im["cntf"] = cnt.reshape(1, S)
    return im, rows


_PROGRAMS = {}


def get_program(sA, sB, **kw):
    key = (sA, sB, tuple(sorted(kw.items())))
    if key not in _PROGRAMS:
        _PROGRAMS[key] = build_program(sA=sA, sB=sB, **kw)
    return _PROGRAMS[key]


# chosen configuration (see ablations): bf16 values, full-patch slab
# (slab=4) channel-major so the combine multiply is unit-stride (DVE bf16
# fast mode), lg=12 run-packed gathers (6KB descriptor per run of up to 12
# pixels with consecutive patch rows). Descriptor generation is cheap
# (~0.3ns/desc); the costs are ~12ns/descriptor of DMA-pipeline overhead
# plus per-position DVE combine work, balanced around lg=12.
# nqueues=4 regresses (measured); trim=True exhausts sequencer registers.
LGDEF = 12
KGDEF = 2
KDEF = KGDEF * LGDEF
CONFIG = dict(vdt=BF16, lg=LGDEF, cmaj=True, trim=False, k=KDEF, slab=4,
              nqueues=2, gbufs=3)
NP_VDT = np.dtype(mybir.dt.np(BF16))


def plan_shards(grid, k=KDEF, lg=LGDEF):
    """Pack runs for every sample and derive the shared slot counts."""
    kg = k // lg
    rslot = P * kg
    packs = []
    for n in range(grid.shape[0]):
        mB = _host_yhalf(grid[n], H, W)
        rel = _host_rel(grid[n], H, W)
        idsA, idsB = np.flatnonzero(~mB), np.flatnonzero(mB)
        packs.append((_pack_runs(idsA, rel[idsA], lg),
                      _pack_runs(idsB, rel[idsB], lg)))
    sA = max(-(-len(pA) // rslot) for pA, _ in packs)
    sB = max(-(-len(pB) // rslot) for _, pB in packs)
    return packs, sA, sB


def kernel(x, grid):
    x = np.asarray(x, np.float32)
    grid = np.asarray(grid, np.float32)
    assert x.shape == (N, C, H, W) and grid.shape == (N, H, W, 2)

    packs, sA, sB = plan_shards(grid)
    nc = get_program(sA, sB, **CONFIG)

    in_maps, rowmaps = [], []
    for n in range(N):
        im, rows = prep_core(x[n], grid[n], sA, sB, k=KDEF, np_vdt=NP_VDT,
                             trim=CONFIG["trim"], lg=LGDEF, packs=packs[n],
                             cmaj=CONFIG["cmaj"], slab=CONFIG["slab"])
        in_maps.append(im)
        rowmaps.append(rows)

    res = bass_utils.run_bass_kernel_spmd(nc, in_maps, core_ids=list(range(N)))

    out = np.empty((N, C, H, W), np.float32)
    for n in range(N):
        od = np.asarray(res.results[n]["outdev"]).astype(np.float32)
        od = od.reshape(P * (sA + sB) * KDEF, C)
        out[n] = od[rowmaps[n]].T.reshape(C, H, W)
    return out

